# revision 20
# baseline (speedup 1.0000x reference)
"""Multi-head attention (qkv pointwise-conv projection + softmax attention)
on 8 Trainium2 NeuronCores.

Problem shapes (hardcoded):
    x:     [B=4, D=512, L=2048] f32
    w_qkv: [3*D=1536, D=512]    f32
    out:   [B, D, L]            f32

Sharding: 2 cores per batch element; each core owns 4 of the 8 heads
(tensor-parallel on the qkv output channels). Core c -> batch c//2,
head group c%2 (heads 4*(c%2) .. 4*(c%2)+3).

Per-core kernel (all in bf16 compute, f32 accumulate):
    Q/K proj:  q[o,l] = sum_d w[o,d] x[d,l]   (layout [head_dim, L])
    V proj  :  vT[l,o]                          (layout [L, head_dim])
               vT stored per head with a fused ones-column -> attn@[v|1]
               yields both the weighted values and the softmax denominator.
    scores  :  St[j,i] = sum_d k[d,j] q[d,i]  (two heads packed in the
               128-row PE array via row tiling: head0 partitions 0-63,
               head1 partitions 64-127)
    softmax :  exp on ScalarE (scale folded into the activation), no max
               subtraction (scores are O(1) by construction)
    attn@v  :  O[d(+den),i] accumulated over j blocks in PSUM
    norm    :  O[d,i] * broadcast(1/den[i])  (broadcast via K=1 matmul)
"""

import os
import numpy as np

B, D, L, H = 4, 512, 2048, 8
HD = D // H  # 64
N_CORES = 8
SCALE = float(D) ** -0.5

# module-level knobs for test.py; harness uses defaults
TRACE = False
LAST_RESULTS = None

_COMPILED = {}


def _build_nc():
    from contextlib import ExitStack

    import concourse.bass as bass
    import concourse.mybir as mybir
    import concourse.tile as tile
    from concourse.bacc import Bacc

    F32 = mybir.dt.float32
    BF16 = mybir.dt.bfloat16
    Exp = mybir.ActivationFunctionType.Exp

    # Bacc (not plain Bass): its finalize() runs the legalization passes that
    # split multi-wait matmuls (walrus MM struct supports only 1 sync wait).
    nc = Bacc("TRN2", target_bir_lowering=False, debug=False)
    # host pre-permuted layouts -> fully contiguous DMA descriptors (4-6KB)
    # x: [p, lc, dc, l'] where d = dc*128+p, l = lc*512+l'
    x_d = nc.dram_tensor("x", [128, 4, 4, 512], BF16, kind="ExternalInput")
    # wT split q|k vs v, grouped by head pair so each pair's weights are one
    # contiguous DMA: [p, pair, dc, o'] with o' = [q_pair(128) | k_pair(128)]
    wqk_d = nc.dram_tensor("wqkT", [128, 2, 4, 256], BF16, kind="ExternalInput")
    wv_d = nc.dram_tensor("wvT", [128, 4, 256], BF16, kind="ExternalInput")
    out_d = nc.dram_tensor("out", [256, L], F32, kind="ExternalOutput")

    NJB = L // 128  # 16 key blocks
    NIC = L // 512  # 4 query chunks

    with ExitStack() as ctx:
        tc = ctx.enter_context(tile.TileContext(nc))
        const = ctx.enter_context(tc.tile_pool(name="const", bufs=1))
        qkp = ctx.enter_context(tc.tile_pool(name="qkp", bufs=1))
        vtp = ctx.enter_context(tc.tile_pool(name="vtp", bufs=1))
        sx = ctx.enter_context(tc.tile_pool(name="sx", bufs=4))
        nrm = ctx.enter_context(tc.tile_pool(name="nrm", bufs=4))
        outp = ctx.enter_context(tc.tile_pool(name="outp", bufs=4))
        drp = ctx.enter_context(tc.tile_pool(name="drp", bufs=4, space="DRAM"))
        ps_st = ctx.enter_context(tc.tile_pool(name="ps_st", bufs=2, space="PSUM"))
        ps_o = ctx.enter_context(tc.tile_pool(name="ps_o", bufs=4, space="PSUM"))

        # ---- PE warmup + load inputs ----
        # ~5 matmuls on zeros keep the PE busy through the input-DMA window
        # so the HAM clock gate opens (1.2 -> 2.4 GHz) before real work.
        scr_sb = const.tile([128, 512], BF16, tag="scr")
        nc.vector.memset(scr_sb[:], 0.0)
        warm_ps = ps_st.tile([128, 1024], F32, tag="st", name="warm")
        for _ in range(4):
            nc.tensor.matmul(warm_ps[:, 0:512], scr_sb[:, 0:128], scr_sb[:])
        # Input DMA is split across BOTH HWDGE rings (SP + Activation): a
        # single ring streams at only ~125-150 GB/s, so the 2.8 MB of input
        # takes ~20 us serialized -- the whole first block ends up DMA-paced.
        # Two rings + first-use ordering gets the critical tensors (wqk pair
        # 0, x chunk 0, wv) on-chip by ~11 us and everything by ~16 us.
        # The Activation-queue issues all happen before the first ACTIVATE,
        # so they never contend with the exp pipeline.
        wqk_sb = const.tile([128, 2, 4, 256], BF16, tag="wqk")
        wv_sb = const.tile([128, 4, 256], BF16, tag="wv")
        x_sb = const.tile([128, 4, 4, 512], BF16, tag="x")
        # DMA scheduling notes (HW-measured): each DMA pays ~1-2us completion
        # latency on top of its transfer time, and the 16 SDMA engines
        # round-robin across BOTH HWDGE rings' queues at packet granularity,
        # so any concurrently-queued transfer delays the critical ones.
        # Phase 1: the first-scores critical mass (wqk pair0 + x chunk 0,
        # dc-sliced so projection matmuls consume slices as they stream) on
        # ring A with ring B carrying only wv. Phase 2 (x1-3, wqk pair1) is
        # gated behind tiny SBUF-read DMAs that wait on the OTHER ring's
        # phase-1 transfers, keeping phase 2 out of the round-robin until
        # phase 1 has landed.
        gate_sb = const.tile([2, 2], BF16, tag="gate")
        nc.sync.dma_start(out=wqk_sb[:, 0, :, :], in_=wqk_d[:, 0, :, :])
        nc.scalar.dma_start(out=wv_sb[:], in_=wv_d[:])
        for dc in range(4):
            nc.sync.dma_start(out=x_sb[:, 0, dc, :], in_=x_d[:, 0, dc, :])
        # phase gates: ring A waits on ring B's wv; ring B waits on ring A's
        # last x0 slice
        nc.sync.dma_start(out=gate_sb[0:1, 0:2], in_=wv_sb[0:1, 0, 0:2])
        nc.scalar.dma_start(out=gate_sb[1:2, 0:2], in_=x_sb[0:1, 0, 3, 0:2])
        nc.sync.dma_start(out=x_sb[:, 1, 0:2, :], in_=x_d[:, 1, 0:2, :])
        nc.scalar.dma_start(out=x_sb[:, 1, 2:4, :], in_=x_d[:, 1, 2:4, :])
        nc.sync.dma_start(out=x_sb[:, 2, 0:2, :], in_=x_d[:, 2, 0:2, :])
        nc.scalar.dma_start(out=x_sb[:, 2, 2:4, :], in_=x_d[:, 2, 2:4, :])
        nc.sync.dma_start(out=x_sb[:, 3, 0:2, :], in_=x_d[:, 3, 0:2, :])
        nc.scalar.dma_start(out=x_sb[:, 3, 2:4, :], in_=x_d[:, 3, 2:4, :])
        nc.scalar.dma_start(out=wqk_sb[:, 1, :, :], in_=wqk_d[:, 1, :, :])
        ones_sb = const.tile([1, 64], BF16, tag="ones")
        nc.vector.memset(ones_sb[:], 1.0)


        q_sb = [qkp.tile([128, L], BF16, tag=f"q{p}", name=f"q{p}") for p in range(2)]
        k_sb = [qkp.tile([128, L], BF16, tag=f"k{p}", name=f"k{p}") for p in range(2)]
        vt_sb = [vtp.tile([128, 4, 65], BF16, tag=f"vt{jb}", name=f"vt{jb}") for jb in range(NJB)]

        # Projection groups run in 1-bank [128,512] PSUM tiles from the shared
        # "o" pool so they never contend with the exp-feeding st pipeline.
        # Each group is split into two 2-matmul quanta so a single filler
        # slot never steals more than ~1/2 period of PE time (a whole 4-mm
        # group in one period delays the next scores pair and stalls the
        # exp pipeline).
        _qk_ps = {}

        def g_qk_h(p, sec, lc, half):
            # half 0: dc 0-1 (opens PSUM accumulation); half 1: dc 2-3
            # (closes it + copies out)
            def f():
                key = (p, sec, lc)
                if half == 0:
                    _qk_ps[key] = ps_o.tile([128, 512], F32, tag="o", name="projg")
                ps = _qk_ps[key]
                qk = 0 if sec == 0 else 128
                for dc in (2 * half, 2 * half + 1):
                    nc.tensor.matmul(
                        ps[:],
                        wqk_sb[:, p, dc, qk : qk + 128],
                        x_sb[:, lc, dc, :],
                        start=(dc == 0),
                        stop=(dc == 3),
                    )
                if half == 1:
                    dst = q_sb[p] if sec == 0 else k_sb[p]
                    nc.vector.tensor_copy(dst[:, lc * 512 : (lc + 1) * 512], ps[:])
                    del _qk_ps[key]

            return f

        def g_qk(p, sec, lc):
            h0, h1 = g_qk_h(p, sec, lc, 0), g_qk_h(p, sec, lc, 1)

            def f():
                h0()
                h1()

            return f

        def g_vt(jb):
            def f():
                nc.vector.memset(vt_sb[jb][:, :, 64:65], 1.0)
                ps = ps_o.tile([128, 512], F32, tag="o", name="projv")
                for dc in range(4):
                    nc.tensor.matmul(
                        ps[:, 0:256],
                        x_sb[:, jb // 4, dc, (jb % 4) * 128 : (jb % 4 + 1) * 128],
                        wv_sb[:, dc, :],
                        start=(dc == 0),
                        stop=(dc == 3),
                    )
                nc.vector.tensor_copy(
                    vt_sb[jb][:, :, 0:64],
                    ps[:, 0:256].rearrange("par (h e) -> par h e", e=64),
                )

            return f

        def st_mms(p, ic, jb):
            # St[j, i] for both heads of pair p, row-packed in the PE
            st = ps_st.tile([128, 1024], F32, tag="st")
            i0 = ic * 512
            for hp in range(2):
                nc.tensor.matmul(
                    st[:, hp * 512 : (hp + 1) * 512],
                    k_sb[p][hp * 64 : (hp + 1) * 64, jb * 128 : (jb + 1) * 128],
                    q_sb[p][hp * 64 : (hp + 1) * 64, i0 : i0 + 512],
                    start=True,
                    stop=True,
                )
            return st

        # the scores pipeline runs one period ahead ACROSS block boundaries:
        # at the last period of block N we issue block N+1's first scores
        # pair, so the exp stream never sees a boundary bubble (the next
        # block's first exp otherwise waits on last-attnv + scores serially)
        _st_next = [None]

        def attn_block(p, ic, fillers=(), pe_bcast=False, nxt=None):
            # scores+softmax+attn@v for head pair p, query chunk ic (512 wide)
            # fillers: {jb: [callables]} — projection groups interleaved into
            # the loop to fill PE slack without starving ScalarE
            # pe_bcast: broadcast 1/den on the PE (shorter latency chain) —
            # used for the final block where the chain is the kernel tail
            # nxt: (p, ic) of the next block, or None for the last
            fillers = dict(fillers)
            i0 = ic * 512

            o_ps = [ps_o.tile([65, 512], F32, tag="o", name="o_acc") for _ in range(2)]
            st_cur = _st_next[0] if _st_next[0] is not None else st_mms(p, ic, 0)
            for jb in range(NJB):
                se = sx.tile([128, 1024], BF16, tag="se")
                nc.scalar.activation(se[:], st_cur[:], Exp, scale=SCALE)
                if jb + 1 < NJB:
                    st_cur = st_mms(p, ic, jb + 1)
                elif nxt is not None:
                    _st_next[0] = st_mms(nxt[0], nxt[1], 0)
                else:
                    _st_next[0] = None
                for f in fillers.get(jb, ()):
                    f()
                for hp in range(2):
                    nc.tensor.matmul(
                        o_ps[hp][:],
                        vt_sb[jb][:, 2 * p + hp, :],
                        se[:, hp * 512 : (hp + 1) * 512],
                        start=(jb == 0),
                        stop=(jb == NJB - 1),
                    )
            if pe_bcast:
                # Kernel-tail normalization: every op here is serial wall
                # time, so fuse the two heads into single wide DVE ops and
                # broadcast the raw denominator FIRST (one bf16 matmul),
                # computing the reciprocal on all 64 partitions in parallel
                # instead of on a single 1-partition row.
                dben = nrm.tile([1, 1024], BF16, tag="denbf")
                for hp in range(2):
                    nc.vector.tensor_copy(
                        dben[:, hp * 512 : (hp + 1) * 512], o_ps[hp][64:65, :]
                    )
                bc_ps = ps_st.tile([128, 1024], F32, tag="st", name="bcast2")
                for hp in range(2):
                    # one matmul per 512-col half: a matmul output cannot
                    # span two PSUM banks (>512 fp32 per partition)
                    nc.tensor.matmul(
                        bc_ps[0:64, hp * 512 : (hp + 1) * 512],
                        ones_sb[:],
                        dben[:, hp * 512 : (hp + 1) * 512],
                        start=True,
                        stop=True,
                    )
                dbc = nrm.tile([64, 1024], F32, tag="dbc")
                nc.vector.tensor_copy(dbc[:], bc_ps[0:64, :])
                rbc2 = nrm.tile([64, 1024], F32, tag="rbc2")
                nc.vector.reciprocal_approx_fast(out=rbc2[:], in_=dbc[:])
                ot2 = outp.tile([64, 1024], F32, tag="ot2")
                for hp in range(2):
                    nc.vector.tensor_mul(
                        ot2[:, hp * 512 : (hp + 1) * 512],
                        o_ps[hp][0:64, :],
                        rbc2[:, hp * 512 : (hp + 1) * 512],
                    )
                    hh = 2 * p + hp
                    nc.sync.dma_start(
                        out=out_d[hh * 64 : (hh + 1) * 64, i0 : i0 + 512],
                        in_=ot2[:, hp * 512 : (hp + 1) * 512],
                    )
                return
            # normalize and write out: 1/den on DVE (fast approx), broadcast
            # the row across 64 partitions via a DRAM bounce, multiply.
            for hp in range(2):
                hh = 2 * p + hp
                o = o_ps[hp]
                den_sb = nrm.tile([1, 512], F32, tag="den")
                nc.vector.tensor_copy(den_sb[:], o[64:65, :])
                recip = nrm.tile([1, 512], F32, tag="recip")
                # NB: approx-recip reads garbage from PSUM on HW; SBUF input only
                nc.vector.reciprocal_approx_fast(out=recip[:], in_=den_sb[:])
                rbc = nrm.tile([64, 512], F32, tag="rbc")
                dbounce = drp.tile([1, 512], F32, tag="db", name="db")
                nc.sync.dma_start(out=dbounce[:], in_=recip[:])
                nc.sync.dma_start(
                    out=rbc[:],
                    in_=bass.AP(
                        tensor=dbounce.tensor,
                        offset=dbounce.offset,
                        ap=[[0, 64], [1, 512]],
                    ),
                )
                ot = outp.tile([64, 512], F32, tag="ot")
                nc.vector.tensor_mul(ot[:], o[0:64, :], rbc[:])
                nc.sync.dma_start(
                    out=out_d[hh * 64 : (hh + 1) * 64, i0 : i0 + 512], in_=ot[:]
                )

        # prologue: ONLY the projections the first scores matmul needs
        # (q0 chunk 0 + k0 chunk 0), so the first exp issues as early as
        # possible. vt(0) runs as a period-0 filler (attnv(0) is issued
        # after the fillers, so same-period JIT is safe). x chunks 1-3 are
        # DMA'd here: their transfers overlap the first block and they are
        # first needed at vt(4)/q(0,lc1), several periods in.
        # Constraints: vt[j] by period j of block (0,0); k0 column group m
        # before st(4m); q0 group lc before block (0,lc); q1/k1 before (1,0).
        for f in (g_qk(0, 0, 0), g_qk(0, 256, 0)):
            f()
        attn_block(0, 0, {
            0: [g_vt(0)],
            1: [g_qk_h(0, 256, 1, 0), g_vt(1)],
            2: [g_qk_h(0, 256, 1, 1), g_vt(2)],
            3: [g_vt(3)],
            4: [g_qk_h(0, 256, 2, 0), g_vt(4)],
            5: [g_qk_h(0, 256, 2, 1), g_vt(5)],
            6: [g_vt(6)],
            7: [g_vt(7)],
            8: [g_qk_h(0, 256, 3, 0), g_vt(8)],
            9: [g_qk_h(0, 256, 3, 1), g_vt(9)],
            10: [g_vt(10)],
            11: [g_qk_h(0, 0, 1, 0), g_vt(11)],
            12: [g_qk_h(0, 0, 1, 1), g_vt(12)],
            13: [g_vt(13)],
            14: [g_vt(14)],
            15: [g_vt(15)],
        }, nxt=(0, 1))
        attn_block(0, 1, {
            0: [g_qk_h(0, 0, 2, 0)],
            1: [g_qk_h(0, 0, 2, 1)],
            3: [g_qk_h(0, 0, 3, 0)],
            4: [g_qk_h(0, 0, 3, 1)],
            6: [g_qk_h(1, 0, 0, 0)],
            7: [g_qk_h(1, 0, 0, 1)],
            9: [g_qk_h(1, 256, 0, 0)],
            10: [g_qk_h(1, 256, 0, 1)],
        }, nxt=(0, 2))
        attn_block(0, 2, {
            0: [g_qk_h(1, 0, 1, 0)],
            1: [g_qk_h(1, 0, 1, 1)],
            4: [g_qk_h(1, 256, 1, 0)],
            5: [g_qk_h(1, 256, 1, 1)],
            8: [g_qk_h(1, 0, 2, 0)],
            9: [g_qk_h(1, 0, 2, 1)],
        }, nxt=(0, 3))
        attn_block(0, 3, {
            0: [g_qk_h(1, 256, 2, 0)],
            1: [g_qk_h(1, 256, 2, 1)],
            4: [g_qk_h(1, 0, 3, 0)],
            5: [g_qk_h(1, 0, 3, 1)],
            8: [g_qk_h(1, 256, 3, 0)],
            9: [g_qk_h(1, 256, 3, 1)],
        }, nxt=(1, 0))
        for ic in range(NIC):
            attn_block(
                1, ic, pe_bcast=(ic == 3),
                nxt=(1, ic + 1) if ic < 3 else None,
            )

    nc.finalize()
    return nc


def _get_nc():
    if "nc" not in _COMPILED:
        _COMPILED["nc"] = _build_nc()
    return _COMPILED["nc"]


def _prep_inputs(x, w_qkv):
    """Per-core input maps (host-side sharding)."""
    import ml_dtypes

    bf16 = ml_dtypes.bfloat16
    in_maps = []
    for c in range(N_CORES):
        b, g = c // 2, c % 2
        # x[b] [512, 2048] -> [p, lc, dc, l'] so every DMA descriptor is a
        # 4KB contiguous run
        xb = np.ascontiguousarray(
            x[b].reshape(4, 128, 4, 512).transpose(1, 2, 0, 3)
        ).astype(bf16)
        # w rows for this head group, transposed then laid out [p, pair, dc, o']
        # with o' = [q_pair(128) | k_pair(128)] so each head pair's q+k
        # weights are one contiguous 2KB-per-partition DMA
        pair_blocks = []
        for pp in range(2):
            wq_p = w_qkv[256 * g + 128 * pp : 256 * g + 128 * (pp + 1), :]
            wk_p = w_qkv[512 + 256 * g + 128 * pp : 512 + 256 * g + 128 * (pp + 1), :]
            blk = np.concatenate([wq_p, wk_p], axis=0)  # [256, 512]
            # [256 o', 512 d] -> [d(p,dc) major: [128 p, 4 dc, 256 o']]
            pair_blocks.append(blk.T.reshape(4, 128, 256).transpose(1, 0, 2))
        wqkT = np.ascontiguousarray(
            np.stack(pair_blocks, axis=1)  # [128, 2, 4, 256]
        ).astype(bf16)
        wv_rows = w_qkv[1024 + 256 * g : 1024 + 256 * (g + 1), :]  # [256, 512]
        wvT = np.ascontiguousarray(
            wv_rows.T.reshape(4, 128, 256).transpose(1, 0, 2)
        ).astype(bf16)
        in_maps.append({"x": xb, "wqkT": wqkT, "wvT": wvT})
    return in_maps


def kernel(x, w_qkv):
    global LAST_RESULTS
    from concourse.bass_utils import run_bass_kernel_spmd

    nc = _get_nc()
    in_maps = _prep_inputs(np.asarray(x), np.asarray(w_qkv))
    res = run_bass_kernel_spmd(
        nc, in_maps, core_ids=list(range(N_CORES)), trace=TRACE
    )
    LAST_RESULTS = res
    out = np.empty((B, D, L), dtype=np.float32)
    for c in range(N_CORES):
        b, g = c // 2, c % 2
        out[b, 256 * g : 256 * (g + 1), :] = res.results[c]["out"]
    return out



# revision 22
# speedup vs baseline: 1.0352x; 1.0352x over previous
"""Multi-head attention (qkv pointwise-conv projection + softmax attention)
on 8 Trainium2 NeuronCores.

Problem shapes (hardcoded):
    x:     [B=4, D=512, L=2048] f32
    w_qkv: [3*D=1536, D=512]    f32
    out:   [B, D, L]            f32

Sharding: 2 cores per batch element; each core owns 4 of the 8 heads
(tensor-parallel on the qkv output channels). Core c -> batch c//2,
head group c%2 (heads 4*(c%2) .. 4*(c%2)+3).

Per-core kernel (all in bf16 compute, f32 accumulate):
    Q/K proj:  q[o,l] = sum_d w[o,d] x[d,l]   (layout [head_dim, L])
    V proj  :  vT[l,o]                          (layout [L, head_dim])
               vT stored per head with a fused ones-column -> attn@[v|1]
               yields both the weighted values and the softmax denominator.
    scores  :  St[j,i] = sum_d k[d,j] q[d,i]  (two heads packed in the
               128-row PE array via row tiling: head0 partitions 0-63,
               head1 partitions 64-127)
    softmax :  exp on ScalarE (scale folded into the activation), no max
               subtraction (scores are O(1) by construction)
    attn@v  :  O[d(+den),i] accumulated over j blocks in PSUM
    norm    :  O[d,i] * broadcast(1/den[i])  (broadcast via K=1 matmul)
"""

import os
import numpy as np

B, D, L, H = 4, 512, 2048, 8
HD = D // H  # 64
N_CORES = 8
SCALE = float(D) ** -0.5

# module-level knobs for test.py; harness uses defaults
TRACE = False
LAST_RESULTS = None

_COMPILED = {}


def _build_nc():
    from contextlib import ExitStack

    import concourse.bass as bass
    import concourse.mybir as mybir
    import concourse.tile as tile
    from concourse.bacc import Bacc

    F32 = mybir.dt.float32
    BF16 = mybir.dt.bfloat16
    Exp = mybir.ActivationFunctionType.Exp

    # Bacc (not plain Bass): its finalize() runs the legalization passes that
    # split multi-wait matmuls (walrus MM struct supports only 1 sync wait).
    nc = Bacc("TRN2", target_bir_lowering=False, debug=False)
    # host pre-permuted layouts -> fully contiguous DMA descriptors (4-6KB)
    # x: [p, lc, dc, l'] where d = dc*128+p, l = lc*512+l'
    x_d = nc.dram_tensor("x", [128, 4, 4, 512], BF16, kind="ExternalInput")
    # wT split q|k vs v, grouped by head pair so each pair's weights are one
    # contiguous DMA: [p, pair, dc, o'] with o' = [q_pair(128) | k_pair(128)]
    wqk_d = nc.dram_tensor("wqkT", [128, 2, 4, 256], BF16, kind="ExternalInput")
    wv_d = nc.dram_tensor("wvT", [128, 4, 256], BF16, kind="ExternalInput")
    out_d = nc.dram_tensor("out", [256, L], F32, kind="ExternalOutput")

    NJB = L // 128  # 16 key blocks
    NIC = L // 512  # 4 query chunks

    with ExitStack() as ctx:
        tc = ctx.enter_context(tile.TileContext(nc))
        const = ctx.enter_context(tc.tile_pool(name="const", bufs=1))
        qkp = ctx.enter_context(tc.tile_pool(name="qkp", bufs=1))
        vtp = ctx.enter_context(tc.tile_pool(name="vtp", bufs=1))
        sx = ctx.enter_context(tc.tile_pool(name="sx", bufs=4))
        nrm = ctx.enter_context(tc.tile_pool(name="nrm", bufs=4))
        outp = ctx.enter_context(tc.tile_pool(name="outp", bufs=4))
        drp = ctx.enter_context(tc.tile_pool(name="drp", bufs=4, space="DRAM"))
        ps_st = ctx.enter_context(tc.tile_pool(name="ps_st", bufs=2, space="PSUM"))
        ps_o = ctx.enter_context(tc.tile_pool(name="ps_o", bufs=4, space="PSUM"))

        # ---- PE warmup + load inputs ----
        # ~5 matmuls on zeros keep the PE busy through the input-DMA window
        # so the HAM clock gate opens (1.2 -> 2.4 GHz) before real work.
        scr_sb = const.tile([128, 512], BF16, tag="scr")
        nc.vector.memset(scr_sb[:], 0.0)
        warm_ps = ps_st.tile([128, 1024], F32, tag="st", name="warm")
        for _ in range(4):
            nc.tensor.matmul(warm_ps[:, 0:512], scr_sb[:, 0:128], scr_sb[:])
        # Input DMA is split across BOTH HWDGE rings (SP + Activation): a
        # single ring streams at only ~125-150 GB/s, so the 2.8 MB of input
        # takes ~20 us serialized -- the whole first block ends up DMA-paced.
        # Two rings + first-use ordering gets the critical tensors (wqk pair
        # 0, x chunk 0, wv) on-chip by ~11 us and everything by ~16 us.
        # The Activation-queue issues all happen before the first ACTIVATE,
        # so they never contend with the exp pipeline.
        wqk_sb = const.tile([128, 2, 4, 256], BF16, tag="wqk")
        wv_sb = const.tile([128, 4, 256], BF16, tag="wv")
        x_sb = const.tile([128, 4, 4, 512], BF16, tag="x")
        # DMA scheduling notes (HW-measured): each DMA pays ~1-2us completion
        # latency on top of its transfer time, and the 16 SDMA engines
        # round-robin across BOTH HWDGE rings' queues at packet granularity,
        # so any concurrently-queued transfer delays the critical ones.
        # Phase 1: the first-scores critical mass (wqk pair0 + x chunk 0,
        # dc-sliced so projection matmuls consume slices as they stream) on
        # ring A with ring B carrying only wv. Phase 2 (x1-3, wqk pair1) is
        # gated behind tiny SBUF-read DMAs that wait on the OTHER ring's
        # phase-1 transfers, keeping phase 2 out of the round-robin until
        # phase 1 has landed.
        nc.sync.dma_start(out=wqk_sb[:, 0, :, :], in_=wqk_d[:, 0, :, :])
        for dc in range(4):
            nc.sync.dma_start(out=x_sb[:, 0, dc, :], in_=x_d[:, 0, dc, :])
        nc.sync.dma_start(out=wv_sb[:], in_=wv_d[:])
        for lc in range(1, 4):
            nc.sync.dma_start(out=x_sb[:, lc, :, :], in_=x_d[:, lc, :, :])
        nc.sync.dma_start(out=wqk_sb[:, 1, :, :], in_=wqk_d[:, 1, :, :])
        ones_sb = const.tile([1, 64], BF16, tag="ones")
        nc.vector.memset(ones_sb[:], 1.0)


        q_sb = [qkp.tile([128, L], BF16, tag=f"q{p}", name=f"q{p}") for p in range(2)]
        k_sb = [qkp.tile([128, L], BF16, tag=f"k{p}", name=f"k{p}") for p in range(2)]
        vt_sb = [vtp.tile([128, 4, 65], BF16, tag=f"vt{jb}", name=f"vt{jb}") for jb in range(NJB)]

        # Projection groups run in 1-bank [128,512] PSUM tiles from the shared
        # "o" pool so they never contend with the exp-feeding st pipeline.
        # Each group is split into two 2-matmul quanta so a single filler
        # slot never steals more than ~1/2 period of PE time (a whole 4-mm
        # group in one period delays the next scores pair and stalls the
        # exp pipeline).
        _qk_ps = {}

        def g_qk_h(p, sec, lc, half):
            # half 0: dc 0-1 (opens PSUM accumulation); half 1: dc 2-3
            # (closes it + copies out)
            def f():
                key = (p, sec, lc)
                if half == 0:
                    _qk_ps[key] = ps_o.tile([128, 512], F32, tag="o", name="projg")
                ps = _qk_ps[key]
                qk = 0 if sec == 0 else 128
                for dc in (2 * half, 2 * half + 1):
                    nc.tensor.matmul(
                        ps[:],
                        wqk_sb[:, p, dc, qk : qk + 128],
                        x_sb[:, lc, dc, :],
                        start=(dc == 0),
                        stop=(dc == 3),
                    )
                if half == 1:
                    dst = q_sb[p] if sec == 0 else k_sb[p]
                    nc.vector.tensor_copy(dst[:, lc * 512 : (lc + 1) * 512], ps[:])
                    del _qk_ps[key]

            return f

        def g_qk(p, sec, lc):
            h0, h1 = g_qk_h(p, sec, lc, 0), g_qk_h(p, sec, lc, 1)

            def f():
                h0()
                h1()

            return f

        def g_vt(jb):
            def f():
                nc.vector.memset(vt_sb[jb][:, :, 64:65], 1.0)
                ps = ps_o.tile([128, 512], F32, tag="o", name="projv")
                for dc in range(4):
                    nc.tensor.matmul(
                        ps[:, 0:256],
                        x_sb[:, jb // 4, dc, (jb % 4) * 128 : (jb % 4 + 1) * 128],
                        wv_sb[:, dc, :],
                        start=(dc == 0),
                        stop=(dc == 3),
                    )
                nc.vector.tensor_copy(
                    vt_sb[jb][:, :, 0:64],
                    ps[:, 0:256].rearrange("par (h e) -> par h e", e=64),
                )

            return f

        def st_mms(p, ic, jb):
            # St[j, i] for both heads of pair p, row-packed in the PE
            st = ps_st.tile([128, 1024], F32, tag="st")
            i0 = ic * 512
            for hp in range(2):
                nc.tensor.matmul(
                    st[:, hp * 512 : (hp + 1) * 512],
                    k_sb[p][hp * 64 : (hp + 1) * 64, jb * 128 : (jb + 1) * 128],
                    q_sb[p][hp * 64 : (hp + 1) * 64, i0 : i0 + 512],
                    start=True,
                    stop=True,
                )
            return st

        # the scores pipeline runs one period ahead ACROSS block boundaries:
        # at the last period of block N we issue block N+1's first scores
        # pair, so the exp stream never sees a boundary bubble (the next
        # block's first exp otherwise waits on last-attnv + scores serially)
        _st_next = [None]

        def attn_block(p, ic, fillers=(), pe_bcast=False, nxt=None):
            # scores+softmax+attn@v for head pair p, query chunk ic (512 wide)
            # fillers: {jb: [callables]} — projection groups interleaved into
            # the loop to fill PE slack without starving ScalarE
            # pe_bcast: broadcast 1/den on the PE (shorter latency chain) —
            # used for the final block where the chain is the kernel tail
            # nxt: (p, ic) of the next block, or None for the last
            fillers = dict(fillers)
            i0 = ic * 512

            o_ps = [ps_o.tile([65, 512], F32, tag="o", name="o_acc") for _ in range(2)]
            st_cur = _st_next[0] if _st_next[0] is not None else st_mms(p, ic, 0)
            for jb in range(NJB):
                se = sx.tile([128, 1024], BF16, tag="se")
                nc.scalar.activation(se[:], st_cur[:], Exp, scale=SCALE)
                if jb + 1 < NJB:
                    st_cur = st_mms(p, ic, jb + 1)
                elif nxt is not None:
                    _st_next[0] = st_mms(nxt[0], nxt[1], 0)
                else:
                    _st_next[0] = None
                for f in fillers.get(jb, ()):
                    f()
                for hp in range(2):
                    nc.tensor.matmul(
                        o_ps[hp][:],
                        vt_sb[jb][:, 2 * p + hp, :],
                        se[:, hp * 512 : (hp + 1) * 512],
                        start=(jb == 0),
                        stop=(jb == NJB - 1),
                    )
            if pe_bcast:
                # Kernel-tail normalization: every op here is serial wall
                # time, so fuse the two heads into single wide DVE ops and
                # broadcast the raw denominator FIRST (one bf16 matmul),
                # computing the reciprocal on all 64 partitions in parallel
                # instead of on a single 1-partition row.
                dben = nrm.tile([1, 1024], BF16, tag="denbf")
                for hp in range(2):
                    nc.vector.tensor_copy(
                        dben[:, hp * 512 : (hp + 1) * 512], o_ps[hp][64:65, :]
                    )
                bc_ps = ps_st.tile([128, 1024], F32, tag="st", name="bcast2")
                for hp in range(2):
                    # one matmul per 512-col half: a matmul output cannot
                    # span two PSUM banks (>512 fp32 per partition)
                    nc.tensor.matmul(
                        bc_ps[0:64, hp * 512 : (hp + 1) * 512],
                        ones_sb[:],
                        dben[:, hp * 512 : (hp + 1) * 512],
                        start=True,
                        stop=True,
                    )
                dbc = nrm.tile([64, 1024], F32, tag="dbc")
                nc.vector.tensor_copy(dbc[:], bc_ps[0:64, :])
                rbc2 = nrm.tile([64, 1024], F32, tag="rbc2")
                nc.vector.reciprocal_approx_fast(out=rbc2[:], in_=dbc[:])
                ot2 = outp.tile([64, 1024], F32, tag="ot2")
                for hp in range(2):
                    nc.vector.tensor_mul(
                        ot2[:, hp * 512 : (hp + 1) * 512],
                        o_ps[hp][0:64, :],
                        rbc2[:, hp * 512 : (hp + 1) * 512],
                    )
                    hh = 2 * p + hp
                    nc.sync.dma_start(
                        out=out_d[hh * 64 : (hh + 1) * 64, i0 : i0 + 512],
                        in_=ot2[:, hp * 512 : (hp + 1) * 512],
                    )
                return
            # normalize and write out: 1/den on DVE (fast approx), broadcast
            # the row across 64 partitions via a DRAM bounce, multiply.
            for hp in range(2):
                hh = 2 * p + hp
                o = o_ps[hp]
                den_sb = nrm.tile([1, 512], F32, tag="den")
                nc.vector.tensor_copy(den_sb[:], o[64:65, :])
                recip = nrm.tile([1, 512], F32, tag="recip")
                # NB: approx-recip reads garbage from PSUM on HW; SBUF input only
                nc.vector.reciprocal_approx_fast(out=recip[:], in_=den_sb[:])
                rbc = nrm.tile([64, 512], F32, tag="rbc")
                dbounce = drp.tile([1, 512], F32, tag="db", name="db")
                nc.sync.dma_start(out=dbounce[:], in_=recip[:])
                nc.sync.dma_start(
                    out=rbc[:],
                    in_=bass.AP(
                        tensor=dbounce.tensor,
                        offset=dbounce.offset,
                        ap=[[0, 64], [1, 512]],
                    ),
                )
                ot = outp.tile([64, 512], F32, tag="ot")
                nc.vector.tensor_mul(ot[:], o[0:64, :], rbc[:])
                nc.sync.dma_start(
                    out=out_d[hh * 64 : (hh + 1) * 64, i0 : i0 + 512], in_=ot[:]
                )

        # prologue: ONLY the projections the first scores matmul needs
        # (q0 chunk 0 + k0 chunk 0), so the first exp issues as early as
        # possible. vt(0) runs as a period-0 filler (attnv(0) is issued
        # after the fillers, so same-period JIT is safe). x chunks 1-3 are
        # DMA'd here: their transfers overlap the first block and they are
        # first needed at vt(4)/q(0,lc1), several periods in.
        # Constraints: vt[j] by period j of block (0,0); k0 column group m
        # before st(4m); q0 group lc before block (0,lc); q1/k1 before (1,0).
        for f in (g_qk(0, 0, 0), g_qk(0, 256, 0)):
            f()
        # k0 chunk lc is consumed by st(4*lc), issued at period 4*lc-1; a
        # projection group must be fully ISSUED before that st (the st waits
        # on the group's DVE cast, and the cast waits on the group's PE
        # matmuls -- issuing them after the st would deadlock the queues).
        # Whole groups at their latest legal period minimize the time the PE
        # spends blocked on the x1-3 input DMAs, which land ~19/22/26 us.
        attn_block(0, 0, {
            0: [g_vt(0)],
            1: [g_vt(1)],
            2: [g_qk(0, 256, 1), g_vt(2)],
            3: [g_vt(3)],
            4: [g_vt(4)],
            5: [g_vt(5)],
            6: [g_qk(0, 256, 2), g_vt(6)],
            7: [g_vt(7)],
            8: [g_vt(8)],
            9: [g_vt(9)],
            10: [g_qk(0, 256, 3), g_vt(10)],
            11: [g_qk_h(0, 0, 1, 0), g_vt(11)],
            12: [g_qk_h(0, 0, 1, 1), g_vt(12)],
            13: [g_vt(13)],
            14: [g_vt(14)],
            15: [g_vt(15)],
        }, nxt=(0, 1))
        attn_block(0, 1, {
            0: [g_qk_h(0, 0, 2, 0)],
            1: [g_qk_h(0, 0, 2, 1)],
            3: [g_qk_h(0, 0, 3, 0)],
            4: [g_qk_h(0, 0, 3, 1)],
            6: [g_qk_h(1, 0, 0, 0)],
            7: [g_qk_h(1, 0, 0, 1)],
            9: [g_qk_h(1, 256, 0, 0)],
            10: [g_qk_h(1, 256, 0, 1)],
        }, nxt=(0, 2))
        attn_block(0, 2, {
            0: [g_qk_h(1, 0, 1, 0)],
            1: [g_qk_h(1, 0, 1, 1)],
            4: [g_qk_h(1, 256, 1, 0)],
            5: [g_qk_h(1, 256, 1, 1)],
            8: [g_qk_h(1, 0, 2, 0)],
            9: [g_qk_h(1, 0, 2, 1)],
        }, nxt=(0, 3))
        attn_block(0, 3, {
            0: [g_qk_h(1, 256, 2, 0)],
            1: [g_qk_h(1, 256, 2, 1)],
            4: [g_qk_h(1, 0, 3, 0)],
            5: [g_qk_h(1, 0, 3, 1)],
            8: [g_qk_h(1, 256, 3, 0)],
            9: [g_qk_h(1, 256, 3, 1)],
        }, nxt=(1, 0))
        for ic in range(NIC):
            attn_block(
                1, ic, pe_bcast=(ic == 3),
                nxt=(1, ic + 1) if ic < 3 else None,
            )

    nc.finalize()
    return nc


def _get_nc():
    if "nc" not in _COMPILED:
        _COMPILED["nc"] = _build_nc()
    return _COMPILED["nc"]


def _prep_inputs(x, w_qkv):
    """Per-core input maps (host-side sharding)."""
    import ml_dtypes

    bf16 = ml_dtypes.bfloat16
    in_maps = []
    for c in range(N_CORES):
        b, g = c // 2, c % 2
        # x[b] [512, 2048] -> [p, lc, dc, l'] so every DMA descriptor is a
        # 4KB contiguous run
        xb = np.ascontiguousarray(
            x[b].reshape(4, 128, 4, 512).transpose(1, 2, 0, 3)
        ).astype(bf16)
        # w rows for this head group, transposed then laid out [p, pair, dc, o']
        # with o' = [q_pair(128) | k_pair(128)] so each head pair's q+k
        # weights are one contiguous 2KB-per-partition DMA
        pair_blocks = []
        for pp in range(2):
            wq_p = w_qkv[256 * g + 128 * pp : 256 * g + 128 * (pp + 1), :]
            wk_p = w_qkv[512 + 256 * g + 128 * pp : 512 + 256 * g + 128 * (pp + 1), :]
            blk = np.concatenate([wq_p, wk_p], axis=0)  # [256, 512]
            # [256 o', 512 d] -> [d(p,dc) major: [128 p, 4 dc, 256 o']]
            pair_blocks.append(blk.T.reshape(4, 128, 256).transpose(1, 0, 2))
        wqkT = np.ascontiguousarray(
            np.stack(pair_blocks, axis=1)  # [128, 2, 4, 256]
        ).astype(bf16)
        wv_rows = w_qkv[1024 + 256 * g : 1024 + 256 * (g + 1), :]  # [256, 512]
        wvT = np.ascontiguousarray(
            wv_rows.T.reshape(4, 128, 256).transpose(1, 0, 2)
        ).astype(bf16)
        in_maps.append({"x": xb, "wqkT": wqkT, "wvT": wvT})
    return in_maps


def kernel(x, w_qkv):
    global LAST_RESULTS
    from concourse.bass_utils import run_bass_kernel_spmd

    nc = _get_nc()
    in_maps = _prep_inputs(np.asarray(x), np.asarray(w_qkv))
    res = run_bass_kernel_spmd(
        nc, in_maps, core_ids=list(range(N_CORES)), trace=TRACE
    )
    LAST_RESULTS = res
    out = np.empty((B, D, L), dtype=np.float32)
    for c in range(N_CORES):
        b, g = c // 2, c % 2
        out[b, 256 * g : 256 * (g + 1), :] = res.results[c]["out"]
    return out

